# revision 35
# baseline (speedup 1.0000x reference)
"""Per-patch dynamic conv (nn_DynaMicConv) as a Bass/Tile kernel on 8 TRN2 cores.

Math: for each patch p of a 14x14 grid over a 224x224 image, out[b, :, p] =
W[p] @ patch_pixels[b, p] + bias[p], i.e. 196 independent [64,768] x [768,768]
matmuls. This is DMA-bound: the weight stack is 462 MB and every byte is read
once.

Sharding: patch-parallel. P=196 patches are padded to 200 and split 25 per
core; each core reads only its weight slice, its patch pixels, and writes its
output slice.

Layouts are precomputed on host so every device DMA is a large transfer with
long per-partition contiguous runs (big SDMA descriptors), GROUPS[g] patches
per DMA (per group size s there is one DRAM tensor "ws"/"bs"):
  w<s>  [n_s, 128, s*6*(768+64)]  partition k holds the group's W (rhs chunks
                                  [128,768] per (p,kc), kc-major) followed by
                                  its patch pixels (lhsT chunks [128,64]) --
                                  one DMA delivers a group's W AND x
  b<s>   [n_s, s*768]
  out    [64, PPC*768]            partition b; patch-major columns

Compute per patch: PSUM[64, 768] = sum_kc lhsT_kc.T @ rhs_kc (+ bias via a
ones[1,64] stationary matmul with start=True). Matmul dtype selects the
traffic/accuracy point (MODE): f16 halves DMA bytes vs f32r at ~2x its error.
PSUM -> SBUF copy (cast to the output dtype) on DVE; outputs stage in per-
segment SBUF tiles and store in STORE_CUTS chunks so stores never block the
DVE/PE pipeline. Measured (8 axon TRN2 cores, max over cores): ~101.5us best,
~105-118us typical under neighbor contention; rel err 3.6e-4.
"""

import numpy as np

import concourse.bacc as bacc
import concourse.mybir as mybir
import concourse.tile as tile
from concourse.bass_utils import run_bass_kernel_spmd

B, CIN, IMG, PS, G = 64, 3, 224, 16, 14
P = G * G                 # 196 patches
COUT = 768
K = CIN * PS * PS         # 768 contraction
KCH = K // 128            # 6 k-chunks
NCORES = 8
PPC = (P + NCORES - 1) // NCORES   # 25 patches per core (padded)
PPAD = PPC * NCORES                # 200
# Patches per W/x/bias DMA. Big groups amortize the ~2us per-dma_start
# completion receipt that serializes each HWDGE ring and keep SDMA
# descriptors large (46KB/partition-row); the taper at the end shortens the
# PE tail after the final W byte lands and keeps the PE fed (warm) to the
# finish. W streams on the SP ring; x/bias/output stores ride the ACT ring.
GROUPS = [5, 5, 5, 4, 3, 2, 1]
assert sum(GROUPS) == PPC
# output store split points (patch indices)
STORE_CUTS = [0, 5, 10, 15, 20, 23, PPC]

F32 = mybir.dt.float32

# matmul input dtype: 'f16' (half DMA traffic, ~3e-4 rel err),
# 'f32r' (full fp32 traffic, ~1.5e-4), 'bf16' (half traffic, ~2e-3)
MODE = "f16"
_DTYPES = {
    "f32r": (mybir.dt.float32r, np.float32),
    "f16": (mybir.dt.float16, np.float16),
    "bf16": (mybir.dt.bfloat16, None),  # np dtype resolved lazily via ml_dtypes
}

# store outputs as fp16 (halves store traffic; adds ~1.4e-4 rms rounding)
OUT_F16 = True

# buffer depths (in DMA groups)
WBUFS = 3
XBUFS = 3

# test.py hooks: set TRACE=True before calling kernel() to profile; the
# BassKernelResults of the last run lands in LAST_RESULT.
TRACE = False
TRACE_CORES = [0]
LAST_RESULT = None

_CACHE = {}


def _np_dtype(mode):
    mdt, ndt = _DTYPES[mode]
    if ndt is None:
        import ml_dtypes
        ndt = ml_dtypes.bfloat16
    return mdt, ndt


def _build(mode):
    mdt, _ = _np_dtype(mode)
    odt = mybir.dt.float16 if OUT_F16 else F32
    nc = bacc.Bacc("TRN2", target_bir_lowering=False, debug=False)
    # one DRAM tensor per distinct group size s: [count_s, 128, s*...]
    sizes = sorted(set(GROUPS))
    cnt = {s: GROUPS.count(s) for s in sizes}
    # W and x for a group ride ONE DMA: [.., 128, s*KCH*COUT | s*KCH*B]
    w_d = {s: nc.dram_tensor(f"w{s}", [cnt[s], 128, s * KCH * (COUT + B)], mdt,
                             kind="ExternalInput") for s in sizes}
    b_d = {s: nc.dram_tensor(f"b{s}", [cnt[s], s * COUT], mdt,
                             kind="ExternalInput") for s in sizes}
    ones_d = nc.dram_tensor("ones", [1, B], mdt, kind="ExternalInput")
    o_d = nc.dram_tensor("out", [B, PPC * COUT], odt, kind="ExternalOutput")

    gmax = max(GROUPS)
    with tile.TileContext(nc) as tc:
        with (
            tc.tile_pool(name="const", bufs=1) as cpool,
            tc.tile_pool(name="wp", bufs=WBUFS) as wpool,
            tc.tile_pool(name="bp", bufs=XBUFS) as bpool,
            tc.tile_pool(name="op", bufs=3) as opool,
            tc.tile_pool(name="ps", bufs=3, space="PSUM") as pspool,
        ):
            ones = cpool.tile([1, B], mdt)
            nc.scalar.dma_start(ones[:], ones_d[:])

            sidx = {s: 0 for s in sizes}
            poff = 0
            seg = 0
            oseg = None
            for gi, GPS in enumerate(GROUPS):
                j = sidx[GPS]; sidx[GPS] += 1
                wt = wpool.tile([128, gmax * KCH * (COUT + B)], mdt, tag="w")
                nc.sync.dma_start(wt[:, : GPS * KCH * (COUT + B)], w_d[GPS][j])
                bt = bpool.tile([1, gmax * COUT], mdt, tag="b")
                nc.scalar.dma_start(bt[:, : GPS * COUT], b_d[GPS][j])

                for i in range(GPS):
                    ps1 = pspool.tile([B, 512], F32, tag="ps1", bufs=4)
                    ps2 = pspool.tile([B, 256], F32, tag="ps2")
                    boff = i * COUT
                    nc.tensor.matmul(ps1[:], ones[:], bt[:, boff: boff + 512],
                                     start=True, stop=False)
                    nc.tensor.matmul(ps2[:], ones[:], bt[:, boff + 512: boff + COUT],
                                     start=True, stop=False)
                    xbase = GPS * KCH * COUT
                    for kc in range(KCH):
                        xoff = xbase + (i * KCH + kc) * B
                        woff = (i * KCH + kc) * COUT
                        lhs = wt[:, xoff: xoff + B]
                        last = kc == KCH - 1
                        nc.tensor.matmul(ps1[:], lhs,
                                         wt[:, woff: woff + 512],
                                         start=False, stop=last)
                        nc.tensor.matmul(ps2[:], lhs,
                                         wt[:, woff + 512: woff + COUT],
                                         start=False, stop=last)

                    p = poff + i
                    if p == STORE_CUTS[seg]:
                        nseg = STORE_CUTS[seg + 1] - STORE_CUTS[seg]
                        oseg = opool.tile([B, nseg * COUT], odt, tag="o",
                                          name=f"oseg{seg}")
                    coff = (p - STORE_CUTS[seg]) * COUT
                    nc.vector.tensor_copy(oseg[:, coff: coff + 512], ps1[:])
                    nc.vector.tensor_copy(oseg[:, coff + 512: coff + COUT], ps2[:])
                    if p + 1 == STORE_CUTS[seg + 1]:
                        nc.scalar.dma_start(
                            o_d[:, STORE_CUTS[seg] * COUT: STORE_CUTS[seg + 1] * COUT],
                            oseg[:])
                        seg += 1
                poff += GPS
    nc.compile()
    return nc


def _prep(x, W, b, mode):
    _, ndt = _np_dtype(mode)
    # patch pixels, k-transposed: xp[p, k, b] with k = c*256 + r*16 + s
    xp = (x.reshape(B, CIN, G, PS, G, PS)
           .transpose(2, 4, 1, 3, 5, 0)
           .reshape(P, K, B))
    # -> [P, 128(kpart), KCH, B]
    xr = np.zeros((PPAD, 128, KCH, B), dtype=ndt)
    xr[:P] = xp.reshape(P, KCH, 128, B).transpose(0, 2, 1, 3).astype(ndt)

    # weights: w[p, kpart, kc*COUT + o] = W[p, o, kc*128 + kpart]
    wr = np.zeros((PPAD, 128, KCH * COUT), dtype=ndt)
    wr[:P] = (W.reshape(P, COUT, KCH, 128)
               .transpose(0, 3, 2, 1)
               .reshape(P, 128, KCH * COUT).astype(ndt))

    br = np.zeros((PPAD, COUT), dtype=ndt)
    br[:P] = b.astype(ndt)
    onesv = np.ones((1, B), dtype=ndt)

    sizes = sorted(set(GROUPS))
    in_maps = []
    for c in range(NCORES):
        base = c * PPC
        m = {"ones": onesv}
        packs = {s: ([], []) for s in sizes}
        poff = 0
        for gs in GROUPS:
            pl = slice(base + poff, base + poff + gs)
            # [gs, 128, cols] -> [128, gs*cols], W block then x block
            wg = wr[pl].transpose(1, 0, 2).reshape(128, gs * KCH * COUT)
            xg = (xr[pl].reshape(gs, 128, KCH * B)
                  .transpose(1, 0, 2).reshape(128, gs * KCH * B))
            packs[gs][0].append(np.concatenate([wg, xg], axis=1))
            packs[gs][1].append(br[pl].reshape(gs * COUT))
            poff += gs
        for s in sizes:
            m[f"w{s}"] = np.ascontiguousarray(np.stack(packs[s][0]))
            m[f"b{s}"] = np.ascontiguousarray(np.stack(packs[s][1]))
        in_maps.append(m)
    return in_maps


def kernel(x, W, b):
    global LAST_RESULT
    x = np.ascontiguousarray(np.asarray(x, dtype=np.float32))
    W = np.ascontiguousarray(np.asarray(W, dtype=np.float32))
    b = np.ascontiguousarray(np.asarray(b, dtype=np.float32))
    in_maps = _prep(x, W, b, MODE)
    key = ("nc", MODE, OUT_F16, tuple(GROUPS), WBUFS, XBUFS)
    if key not in _CACHE:
        _CACHE[key] = _build(MODE)
    res = run_bass_kernel_spmd(
        _CACHE[key], in_maps, core_ids=list(range(NCORES)),
        trace=TRACE, trace_cores=TRACE_CORES,
    )
    LAST_RESULT = res
    # assemble: padded patch order is [c*PPC + i]; patches 196..199 are pad
    obig = np.concatenate(
        [res.results[c]["out"].reshape(B, PPC, COUT) for c in range(NCORES)],
        axis=1)                                   # [B, PPAD, COUT]
    out = obig[:, :P].astype(np.float32)          # [B, P, COUT]
    return np.ascontiguousarray(out.transpose(0, 2, 1)).reshape(B, COUT, G, G)


# revision 37
# speedup vs baseline: 1.1259x; 1.1259x over previous
"""Per-patch dynamic conv (nn_DynaMicConv) as a Bass/Tile kernel on 8 TRN2 cores.

Math: for each patch p of a 14x14 grid over a 224x224 image, out[b, :, p] =
W[p] @ patch_pixels[b, p] + bias[p], i.e. 196 independent [64,768] x [768,768]
matmuls. This is DMA-bound: the weight stack is 462 MB and every byte is read
once.

Sharding: patch-parallel. P=196 patches are padded to 200 and split 25 per
core; each core reads only its weight slice, its patch pixels, and writes its
output slice.

Layouts are precomputed on host so every device DMA is a large transfer with
long per-partition contiguous runs (big SDMA descriptors), GROUPS[g] patches
per DMA (per group size s there is one DRAM tensor "ws"/"bs"):
  w<s>  [n_s, 128, s*6*(768+64)]  partition k holds the group's W (rhs chunks
                                  [128,768] per (p,kc), kc-major) followed by
                                  its patch pixels (lhsT chunks [128,64]) --
                                  one DMA delivers a group's W AND x
  b<s>   [n_s, s*768]
  out    [64, PPC*768]            partition b; patch-major columns

Compute per patch: PSUM[64, 768] = sum_kc lhsT_kc.T @ rhs_kc (+ bias via a
ones[1,64] stationary matmul with start=True). Matmul dtype selects the
traffic/accuracy point (MODE): f16 halves DMA bytes vs f32r at ~2x its error.
PSUM -> SBUF copy (cast to the output dtype) on DVE; outputs stage in per-
segment SBUF tiles and store in STORE_CUTS chunks so stores never block the
DVE/PE pipeline. Measured (8 axon TRN2 cores, max over cores): ~101.5us best,
~105-118us typical under neighbor contention; rel err 3.6e-4.
"""

import numpy as np

import concourse.bacc as bacc
import concourse.mybir as mybir
import concourse.tile as tile
from concourse.bass_utils import run_bass_kernel_spmd

B, CIN, IMG, PS, G = 64, 3, 224, 16, 14
P = G * G                 # 196 patches
COUT = 768
K = CIN * PS * PS         # 768 contraction
KCH = K // 128            # 6 k-chunks
NCORES = 8
PPC = (P + NCORES - 1) // NCORES   # 25 patches per core (padded)
PPAD = PPC * NCORES                # 200
# Patches per W/x/bias DMA. Big groups amortize the ~2us per-dma_start
# completion receipt that serializes each HWDGE ring and keep SDMA
# descriptors large (46KB/partition-row); the taper at the end shortens the
# PE tail after the final W byte lands and keeps the PE fed (warm) to the
# finish. W streams on the SP ring; x/bias/output stores ride the ACT ring.
GROUPS = [5, 5, 5, 4, 3, 2, 1]
assert sum(GROUPS) == PPC
# output store split points (patch indices)
STORE_CUTS = [0, 5, 10, 15, 20, 23, PPC]

F32 = mybir.dt.float32

# matmul input dtype: 'f16' (half DMA traffic, ~3e-4 rel err),
# 'f32r' (full fp32 traffic, ~1.5e-4), 'bf16' (half traffic, ~2e-3)
MODE = "f16"
_DTYPES = {
    "f32r": (mybir.dt.float32r, np.float32),
    "f16": (mybir.dt.float16, np.float16),
    "bf16": (mybir.dt.bfloat16, None),  # np dtype resolved lazily via ml_dtypes
}

# store outputs as fp16 (halves store traffic; adds ~1.4e-4 rms rounding)
OUT_F16 = True

# buffer depths (in DMA groups)
WBUFS = 3
XBUFS = 3

# test.py hooks: set TRACE=True before calling kernel() to profile; the
# BassKernelResults of the last run lands in LAST_RESULT.
TRACE = False
TRACE_CORES = [0]
LAST_RESULT = None

_CACHE = {}


def _np_dtype(mode):
    mdt, ndt = _DTYPES[mode]
    if ndt is None:
        import ml_dtypes
        ndt = ml_dtypes.bfloat16
    return mdt, ndt


def _build(mode):
    mdt, _ = _np_dtype(mode)
    odt = mybir.dt.float16 if OUT_F16 else F32
    nc = bacc.Bacc("TRN2", target_bir_lowering=False, debug=False)
    # one DRAM tensor per distinct group size s: [count_s, 128, s*...]
    sizes = sorted(set(GROUPS))
    cnt = {s: GROUPS.count(s) for s in sizes}
    # W and x for a group ride ONE DMA: [.., 128, s*KCH*COUT | s*KCH*B]
    w_d = {s: nc.dram_tensor(f"w{s}", [cnt[s], 128, s * KCH * (COUT + B)], mdt,
                             kind="ExternalInput") for s in sizes}
    b_d = {s: nc.dram_tensor(f"b{s}", [cnt[s], s * COUT], mdt,
                             kind="ExternalInput") for s in sizes}
    ones_d = nc.dram_tensor("ones", [1, B], mdt, kind="ExternalInput")
    o_d = nc.dram_tensor("out", [B, PPC * COUT], odt, kind="ExternalOutput")

    gmax = max(GROUPS)
    with tile.TileContext(nc) as tc:
        with (
            tc.tile_pool(name="const", bufs=1) as cpool,
            tc.tile_pool(name="wp", bufs=WBUFS) as wpool,
            tc.tile_pool(name="bp", bufs=XBUFS) as bpool,
            tc.tile_pool(name="op", bufs=3) as opool,
            tc.tile_pool(name="ps", bufs=3, space="PSUM") as pspool,
        ):
            ones = cpool.tile([1, B], mdt)
            nc.scalar.dma_start(ones[:], ones_d[:])

            sidx = {s: 0 for s in sizes}
            poff = 0
            seg = 0
            oseg = None
            for gi, GPS in enumerate(GROUPS):
                j = sidx[GPS]; sidx[GPS] += 1
                wt = wpool.tile([128, gmax * KCH * (COUT + B)], mdt, tag="w")
                nc.sync.dma_start(wt[:, : GPS * KCH * (COUT + B)], w_d[GPS][j])
                bt = bpool.tile([1, gmax * COUT], mdt, tag="b")
                nc.scalar.dma_start(bt[:, : GPS * COUT], b_d[GPS][j])

                for i in range(GPS):
                    ps1 = pspool.tile([B, 512], F32, tag="ps1", bufs=4)
                    ps2 = pspool.tile([B, 256], F32, tag="ps2")
                    boff = i * COUT
                    nc.tensor.matmul(ps1[:], ones[:], bt[:, boff: boff + 512],
                                     start=True, stop=False)
                    nc.tensor.matmul(ps2[:], ones[:], bt[:, boff + 512: boff + COUT],
                                     start=True, stop=False)
                    xbase = GPS * KCH * COUT
                    for kc in range(KCH):
                        xoff = xbase + (i * KCH + kc) * B
                        woff = (i * KCH + kc) * COUT
                        lhs = wt[:, xoff: xoff + B]
                        last = kc == KCH - 1
                        nc.tensor.matmul(ps1[:], lhs,
                                         wt[:, woff: woff + 512],
                                         start=False, stop=last)
                        nc.tensor.matmul(ps2[:], lhs,
                                         wt[:, woff + 512: woff + COUT],
                                         start=False, stop=last)

                    p = poff + i
                    if p == STORE_CUTS[seg]:
                        nseg = STORE_CUTS[seg + 1] - STORE_CUTS[seg]
                        oseg = opool.tile([B, nseg * COUT], odt, tag="o",
                                          name=f"oseg{seg}")
                    coff = (p - STORE_CUTS[seg]) * COUT
                    nc.vector.tensor_copy(oseg[:, coff: coff + 512], ps1[:])
                    nc.vector.tensor_copy(oseg[:, coff + 512: coff + COUT], ps2[:])
                    if p + 1 == STORE_CUTS[seg + 1]:
                        nc.scalar.dma_start(
                            o_d[:, STORE_CUTS[seg] * COUT: STORE_CUTS[seg + 1] * COUT],
                            oseg[:])
                        seg += 1
                poff += GPS
    nc.compile()
    return nc


def _prep(x, W, b, mode):
    _, ndt = _np_dtype(mode)
    # patch pixels, k-transposed: xp[p, k, b] with k = c*256 + r*16 + s
    xp = (x.reshape(B, CIN, G, PS, G, PS)
           .transpose(2, 4, 1, 3, 5, 0)
           .reshape(P, K, B))
    # -> [P, 128(kpart), KCH, B]
    xr = np.zeros((PPAD, 128, KCH, B), dtype=ndt)
    xr[:P] = xp.reshape(P, KCH, 128, B).transpose(0, 2, 1, 3).astype(ndt)

    # weights: w[p, kpart, kc*COUT + o] = W[p, o, kc*128 + kpart]
    wr = np.zeros((PPAD, 128, KCH * COUT), dtype=ndt)
    wr[:P] = (W.reshape(P, COUT, KCH, 128)
               .transpose(0, 3, 2, 1)
               .reshape(P, 128, KCH * COUT).astype(ndt))

    br = np.zeros((PPAD, COUT), dtype=ndt)
    br[:P] = b.astype(ndt)
    onesv = np.ones((1, B), dtype=ndt)

    sizes = sorted(set(GROUPS))
    in_maps = []
    for c in range(NCORES):
        base = c * PPC
        m = {"ones": onesv}
        packs = {s: ([], []) for s in sizes}
        poff = 0
        for gs in GROUPS:
            pl = slice(base + poff, base + poff + gs)
            # [gs, 128, cols] -> [128, gs*cols], W block then x block
            wg = wr[pl].transpose(1, 0, 2).reshape(128, gs * KCH * COUT)
            xg = (xr[pl].reshape(gs, 128, KCH * B)
                  .transpose(1, 0, 2).reshape(128, gs * KCH * B))
            packs[gs][0].append(np.concatenate([wg, xg], axis=1))
            packs[gs][1].append(br[pl].reshape(gs * COUT))
            poff += gs
        for s in sizes:
            m[f"w{s}"] = np.ascontiguousarray(np.stack(packs[s][0]))
            m[f"b{s}"] = np.ascontiguousarray(np.stack(packs[s][1]))
        in_maps.append(m)
    return in_maps


def kernel(x, W, b):
    global LAST_RESULT
    x = np.ascontiguousarray(np.asarray(x, dtype=np.float32))
    W = np.ascontiguousarray(np.asarray(W, dtype=np.float32))
    b = np.ascontiguousarray(np.asarray(b, dtype=np.float32))
    in_maps = _prep(x, W, b, MODE)
    key = ("nc", MODE, OUT_F16, tuple(GROUPS), WBUFS, XBUFS)
    if key not in _CACHE:
        _CACHE[key] = _build(MODE)
    res = run_bass_kernel_spmd(
        _CACHE[key], in_maps, core_ids=list(range(NCORES)),
        trace=TRACE, trace_cores=TRACE_CORES,
    )
    LAST_RESULT = res
    # assemble: padded patch order is [c*PPC + i]; patches 196..199 are pad
    obig = np.concatenate(
        [res.results[c]["out"].reshape(B, PPC, COUT) for c in range(NCORES)],
        axis=1)                                   # [B, PPAD, COUT]
    out = obig[:, :P].astype(np.float32)          # [B, P, COUT]
    return np.ascontiguousarray(out.transpose(0, 2, 1)).reshape(B, COUT, G, G)
